# revision 14
# baseline (speedup 1.0000x reference)
"""Trainium2 Bass kernel for nn_CustomLinear (learned-twiddle butterfly net).

Math: reference pads x [2048,4096] to [2048,8192], half-swaps (XOR N/2), then
13 radix-2 butterfly stages with learned twiddles.  After the half-swap the
lower half of each row is zero, so the transform reduces to a 4096-point
network on the nonzero part followed by out = [t, -t].

This version:
  - stages 1..9 folded into one PE matmul phase: out blocks grouped into 4
    parity classes q = j mod 4, each out block = sum of 4 fp16 [128,128]
    complex matmuls over the 4 input blocks of its group.
  - stages 10..12: elementwise butterflies in fp16, transposed layout
    [e=partition, (m,r)=free]; twiddles are per-partition scalars (st10/st11)
    or broadcast fp16 tiles (st12).
  - stage 13 + un-transpose fused on PE: transpose-matmuls with DIAGONAL fp16
    weight tiles (diag(c_j), diag(s_j)) accumulate t^T = (C z_r - S z_i)^T
    and (S z_r + C z_i)^T directly into PSUM (fp16), 1 cycle/row.
  - interleave re/im + negate (-t half) via strided PSUM->SBUF copies, then
    contiguous DMA to HBM.

Sharding: pure data parallel, batch 2048 -> 8 cores x 256 rows.
"""
import numpy as np
from contextlib import ExitStack

import concourse.bacc as bacc
import concourse.mybir as mybir
from concourse.tile import TileContext
from concourse.bass_utils import run_bass_kernel_spmd

N = 8192
B = 2048
IN_F = 4096
NCORES = 8
B_CORE = B // NCORES          # 256 rows per core
NTILES = B_CORE // 128        # 2 row-tiles of 128 rows
F32 = mybir.dt.float32
F16 = mybir.dt.float16

# cwf (fp32 [128,160]) column layout
_ID = 0                        # identity 128 cols
_S10 = 128                     # st10: 128 + q*2 + {c,s}
_S11 = 136                     # st11: 136 + (q*2+u)*2 + {c,s}
CWF_W = 160
# dgt (fp16 [128, 12288]): T13 broadcast tiles (q*2+{c,s})*1024 + m*128 ;
# T12 broadcast tiles at 8192 + (q*2+cs)*512
_T12 = 8192
DGT_W = 12288
# wts (fp16 [128,4096]): ((q*4+jp)*2+{re,im})*128
WTS_W = 4096

_CACHE = {}


def _stage_tw(s, w):
    step = 1 << s
    half = step >> 1
    k = np.arange(half) * (N // step)
    ang = (-2.0 * np.pi / N) * k.astype(np.float64) * w[k].astype(np.float64)
    return np.exp(1j * ang)


def _host_consts(w):
    M = np.eye(128, dtype=np.complex128)
    for s in range(1, 8):
        step = 1 << s
        half = step >> 1
        tw = _stage_tw(s, w)
        Bm = np.zeros((step, step), np.complex128)
        Bm[:half, :half] = np.eye(half)
        Bm[:half, half:] = np.diag(tw)
        Bm[half:, :half] = np.eye(half)
        Bm[half:, half:] = -np.diag(tw)
        M = np.kron(np.eye(128 // step), Bm) @ M

    tw8, tw9, tw10, tw11, tw12, tw13 = [_stage_tw(s, w) for s in range(8, 14)]
    e = np.arange(128)
    T8 = tw8[e]
    T9 = {jr: tw9[jr * 128 + e] for jr in (0, 1)}

    wts = np.zeros((128, WTS_W), np.float16)
    for q in range(4):
        jr9 = q % 2
        s9 = 1.0 if q < 2 else -1.0
        s8 = 1.0 if q % 2 == 0 else -1.0
        Ws = [M,
              s8 * T8[:, None] * M,
              s9 * T9[jr9][:, None] * M,
              s8 * s9 * (T9[jr9] * T8)[:, None] * M]
        for jp in range(4):
            WT = Ws[jp].T          # lhsT[e_in, e_out]
            c0 = ((q * 4 + jp) * 2) * 128
            wts[:, c0:c0 + 128] = WT.real.astype(np.float16)
            wts[:, c0 + 128:c0 + 256] = WT.imag.astype(np.float16)

    dgt = np.zeros((128, DGT_W), np.float16)
    for q in range(4):
        for m in range(8):
            d = tw13[(q + 4 * m) * 128 + e]
            c0 = (q * 2) * 1024 + m * 128
            dgt[:, c0:c0 + 128] = \
                np.repeat(d.real.astype(np.float16)[:, None], 128, 1)
            c0 = (q * 2 + 1) * 1024 + m * 128
            dgt[:, c0:c0 + 128] = \
                np.repeat(d.imag.astype(np.float16)[:, None], 128, 1)
    for q in range(4):
        for m in range(4):
            t = tw12[(q + 4 * m) * 128 + e]
            dgt[:, _T12 + (q * 2) * 512 + m * 128:
                _T12 + (q * 2) * 512 + (m + 1) * 128] = \
                np.repeat(t.real.astype(np.float16)[:, None], 128, 1)
            dgt[:, _T12 + (q * 2 + 1) * 512 + m * 128:
                _T12 + (q * 2 + 1) * 512 + (m + 1) * 128] = \
                np.repeat(t.imag.astype(np.float16)[:, None], 128, 1)

    cwf = np.zeros((128, CWF_W), np.float32)
    cwf[:, _ID:_ID + 128] = np.eye(128, dtype=np.float32)
    for q in range(4):
        t = tw10[q * 128 + e]
        cwf[:, _S10 + 2 * q] = t.real.astype(np.float32)
        cwf[:, _S10 + 2 * q + 1] = t.imag.astype(np.float32)
        for u in (0, 1):
            t = tw11[(q + 4 * u) * 128 + e]
            cwf[:, _S11 + (q * 2 + u) * 2] = t.real.astype(np.float32)
            cwf[:, _S11 + (q * 2 + u) * 2 + 1] = t.imag.astype(np.float32)
    return wts, dgt, cwf


def _build_program():
    nc = bacc.Bacc("TRN2", target_bir_lowering=False, debug=False)
    x_d = nc.dram_tensor("x", [B_CORE, IN_F], F32, kind="ExternalInput").ap()
    wts_d = nc.dram_tensor("wts", [128, WTS_W], F16, kind="ExternalInput").ap()
    dgt_d = nc.dram_tensor("dgt", [128, DGT_W], F16, kind="ExternalInput").ap()
    cwf_d = nc.dram_tensor("cwf", [128, CWF_W], F32, kind="ExternalInput").ap()
    y_d = nc.dram_tensor("y", [B_CORE, 2 * N], F32, kind="ExternalOutput").ap()

    AL = mybir.AluOpType

    with TileContext(nc) as tc, ExitStack() as ctx:
        cpool = ctx.enter_context(tc.tile_pool(name="const", bufs=1))
        xpool = ctx.enter_context(tc.tile_pool(name="xin", bufs=2))
        xtpool = ctx.enter_context(tc.tile_pool(name="xt", bufs=2))
        zpool = ctx.enter_context(tc.tile_pool(name="z", bufs=2))
        spool = ctx.enter_context(tc.tile_pool(name="scr", bufs=4))
        opool = ctx.enter_context(tc.tile_pool(name="out", bufs=2))
        ps_t = ctx.enter_context(tc.tile_pool(name="ps_t", bufs=2, space="PSUM"))
        ps_z = ctx.enter_context(tc.tile_pool(name="ps_z", bufs=4, space="PSUM"))
        ps_o = ctx.enter_context(tc.tile_pool(name="ps_o", bufs=2, space="PSUM"))

        wts = cpool.tile([128, WTS_W], F16)
        dgt = cpool.tile([128, DGT_W], F16)
        cwf = cpool.tile([128, CWF_W], F32)
        nc.sync.dma_start(cwf[:], cwf_d[:])
        ident = cwf[:, _ID:_ID + 128]
        idf16 = cpool.tile([128, 128], F16)

        xins = []
        for ti in range(NTILES):
            r0 = ti * 128
            xin = xpool.tile([128, IN_F], F32, name=f"xin{ti}", tag="xin")
            nc.sync.dma_start(xin[:, :2048], x_d[r0:r0 + 128, :2048])
            nc.sync.dma_start(xin[:, 2048:], x_d[r0:r0 + 128, 2048:])
            xins.append(xin)
            if ti == 0:
                nc.sync.dma_start(wts[:], wts_d[:])
        nc.sync.dma_start(dgt[:], dgt_d[:])
        nc.scalar.copy(idf16[:], ident)

        def wtile(q, jp, cs):
            c0 = ((q * 4 + jp) * 2 + cs) * 128
            return wts[:, c0:c0 + 128]

        def t13t(q, cs):
            c0 = (q * 2 + cs) * 1024
            return dgt[:, c0:c0 + 1024]

        def t12t(q, cs):
            c0 = _T12 + (q * 2 + cs) * 512
            return dgt[:, c0:c0 + 512]

        # ---- front-load PE work for both tiles: transpose + phase A ----
        ZR, ZI = [], []
        for ti in range(NTILES):
            xin = xins[ti]
            xt = xtpool.tile([128, IN_F], F16, name=f"xt{ti}", tag="xt")
            for bq in range(8):
                pt = ps_t.tile([128, 512], F32, tag="pt")
                for k in range(4):
                    j = bq * 4 + k
                    nc.tensor.transpose(pt[:, k * 128:(k + 1) * 128],
                                        xin[:, j * 128:(j + 1) * 128], ident)
                if bq % 4 == 3:
                    nc.vector.tensor_copy(xt[:, bq * 512:(bq + 1) * 512],
                                          pt[:])
                else:
                    nc.scalar.copy(xt[:, bq * 512:(bq + 1) * 512], pt[:])

            xtv = xt[:].rearrange("p (g v c) -> p g v c", g=8, v=4, c=128)
            zr = [None] * 4
            zi = [None] * 4
            for q in range(4):
                zr[q] = zpool.tile([128, 1024], F16, name=f"zr{ti}{q}",
                                   tag=f"zr{q}")
                zi[q] = zpool.tile([128, 1024], F16, name=f"zi{ti}{q}",
                                   tag=f"zi{q}")
                for h in range(2):
                    sl = slice(h * 512, (h + 1) * 512)
                    for cs, dst in ((0, zr[q]), (1, zi[q])):
                        pz = ps_z.tile([128, 512], F32, tag="pz")
                        for jp in range(4):
                            rhs = xtv[:, 4 * h:4 * h + 4, jp, :]
                            nc.tensor.matmul(pz[:], wtile(q, jp, cs), rhs,
                                             start=(jp == 0), stop=(jp == 3))
                        if (q + h + cs) % 4 == 3:
                            nc.vector.tensor_copy(dst[:, sl], pz[:])
                        else:
                            nc.scalar.copy(dst[:, sl], pz[:])
            ZR.append(zr)
            ZI.append(zi)

        # ---- stages 10..13: stage-major, interleaved over (tile, class) ----
        def st10(ti, q):
            zr, zi = ZR[ti][q], ZI[ti][q]
            c10 = cwf[:, _S10 + 2 * q:_S10 + 2 * q + 1]
            s10 = cwf[:, _S10 + 2 * q + 1:_S10 + 2 * q + 2]
            vr = zr[:].rearrange("p (t two c) -> p t two c", two=2, c=128)
            vi = zi[:].rearrange("p (t two c) -> p t two c", two=2, c=128)
            lo_r, hi_r = vr[:, :, 0, :], vr[:, :, 1, :]
            lo_i, hi_i = vi[:, :, 0, :], vi[:, :, 1, :]
            m1 = spool.tile([128, 512], F16, tag="m1")
            m2 = spool.tile([128, 512], F16, tag="m2")
            tr = spool.tile([128, 512], F16, tag="tr")
            tmi = spool.tile([128, 512], F16, tag="tmi")
            w1 = m1[:].rearrange("p (t c) -> p t c", t=4, c=128)
            w2 = m2[:].rearrange("p (t c) -> p t c", t=4, c=128)
            wr = tr[:].rearrange("p (t c) -> p t c", t=4, c=128)
            wi = tmi[:].rearrange("p (t c) -> p t c", t=4, c=128)
            nc.scalar.mul(w1, hi_i, s10)
            nc.scalar.mul(w2, hi_i, c10)
            nc.vector.scalar_tensor_tensor(
                wr, hi_r, c10, w1, op0=AL.mult, op1=AL.subtract)
            nc.vector.scalar_tensor_tensor(
                wi, hi_r, s10, w2, op0=AL.mult, op1=AL.add)
            nc.gpsimd.tensor_tensor(hi_r, lo_r, wr, op=AL.subtract)
            nc.vector.tensor_tensor(hi_i, lo_i, wi, op=AL.subtract)
            nc.vector.tensor_tensor(lo_r, lo_r, wr, op=AL.add)
            nc.vector.tensor_tensor(lo_i, lo_i, wi, op=AL.add)

        def st11(ti, q):
            zr, zi = ZR[ti][q], ZI[ti][q]
            vr = zr[:].rearrange("p (g v c) -> p g v c", g=2, v=4, c=128)
            vi = zi[:].rearrange("p (g v c) -> p g v c", g=2, v=4, c=128)
            for u in (0, 1):
                c11 = cwf[:, _S11 + (q * 2 + u) * 2:
                          _S11 + (q * 2 + u) * 2 + 1]
                s11 = cwf[:, _S11 + (q * 2 + u) * 2 + 1:
                          _S11 + (q * 2 + u) * 2 + 2]
                lo_r, hi_r = vr[:, :, u, :], vr[:, :, u + 2, :]
                lo_i, hi_i = vi[:, :, u, :], vi[:, :, u + 2, :]
                m1 = spool.tile([128, 256], F16, tag="m1b")
                m2 = spool.tile([128, 256], F16, tag="m2b")
                tr = spool.tile([128, 256], F16, tag="trb")
                tmi = spool.tile([128, 256], F16, tag="timb")
                w1 = m1[:].rearrange("p (g c) -> p g c", g=2, c=128)
                w2 = m2[:].rearrange("p (g c) -> p g c", g=2, c=128)
                wr = tr[:].rearrange("p (g c) -> p g c", g=2, c=128)
                wi = tmi[:].rearrange("p (g c) -> p g c", g=2, c=128)
                nc.scalar.mul(w1, hi_i, s11)
                nc.scalar.mul(w2, hi_i, c11)
                nc.vector.scalar_tensor_tensor(
                    wr, hi_r, c11, w1, op0=AL.mult, op1=AL.subtract)
                nc.vector.scalar_tensor_tensor(
                    wi, hi_r, s11, w2, op0=AL.mult, op1=AL.add)
                nc.gpsimd.tensor_tensor(hi_r, lo_r, wr, op=AL.subtract)
                nc.gpsimd.tensor_tensor(hi_i, lo_i, wi, op=AL.subtract)
                nc.vector.tensor_tensor(lo_r, lo_r, wr, op=AL.add)
                nc.vector.tensor_tensor(lo_i, lo_i, wi, op=AL.add)

        def st12(ti, q):
            zr, zi = ZR[ti][q], ZI[ti][q]
            lo_r, hi_r = zr[:, :512], zr[:, 512:]
            lo_i, hi_i = zi[:, :512], zi[:, 512:]
            u1 = spool.tile([128, 512], F16, tag="u1")
            u2 = spool.tile([128, 512], F16, tag="u2")
            tr = spool.tile([128, 512], F16, tag="tr2")
            tmi = spool.tile([128, 512], F16, tag="tim2")
            nc.vector.tensor_tensor(u1[:], hi_r, t12t(q, 0), op=AL.mult)
            nc.vector.tensor_tensor(u2[:], hi_i, t12t(q, 1), op=AL.mult)
            nc.vector.tensor_tensor(tr[:], u1[:], u2[:], op=AL.subtract)
            nc.gpsimd.tensor_tensor(u1[:], hi_r, t12t(q, 1), op=AL.mult)
            nc.vector.tensor_tensor(u2[:], hi_i, t12t(q, 0), op=AL.mult)
            nc.gpsimd.tensor_tensor(tmi[:], u1[:], u2[:], op=AL.add)
            nc.gpsimd.tensor_tensor(hi_r, lo_r, tr[:], op=AL.subtract)
            nc.vector.tensor_tensor(hi_i, lo_i, tmi[:], op=AL.subtract)
            nc.vector.tensor_tensor(lo_r, lo_r, tr[:], op=AL.add)
            nc.vector.tensor_tensor(lo_i, lo_i, tmi[:], op=AL.add)

        def st13(ti, q):
            # in place: zr <- Re(tw13*z), zi <- Im(tw13*z)
            zr, zi = ZR[ti][q], ZI[ti][q]
            u1 = spool.tile([128, 1024], F16, tag="su1")
            u2 = spool.tile([128, 1024], F16, tag="su2")
            u3 = spool.tile([128, 1024], F16, tag="su3")
            u4 = spool.tile([128, 1024], F16, tag="su4")
            nc.vector.tensor_tensor(u1[:], zr[:], t13t(q, 0), op=AL.mult)
            nc.vector.tensor_tensor(u2[:], zi[:], t13t(q, 1), op=AL.mult)
            nc.gpsimd.tensor_tensor(u3[:], zr[:], t13t(q, 1), op=AL.mult)
            nc.vector.tensor_tensor(u4[:], zi[:], t13t(q, 0), op=AL.mult)
            nc.vector.tensor_tensor(zr[:], u1[:], u2[:], op=AL.subtract)
            nc.vector.tensor_tensor(zi[:], u3[:], u4[:], op=AL.add)

        for stage in (st10, st11, st12, st13):
            for ti in range(NTILES):
                for q in range(4):
                    stage(ti, q)

        # ---- transpose back (fp16); interleave + negate; DMA out ----
        for ti in range(NTILES):
            r0 = ti * 128
            tre, tim = ZR[ti], ZI[ti]
            for u in range(8):
                po = ps_o.tile([128, 1024], F16, tag="po")
                pr, pi = po[:, :512], po[:, 512:]
                for q in range(4):
                    ms = slice(u * 128, (u + 1) * 128)
                    os_ = slice(q * 128, (q + 1) * 128)
                    nc.tensor.transpose(pr[:, os_], tre[q][:, ms], idf16[:])
                    nc.tensor.transpose(pi[:, os_], tim[q][:, ms], idf16[:])
                op_ = opool.tile([128, 1024], F32, tag="op")
                on_ = opool.tile([128, 1024], F32, tag="on")
                vp = op_[:].rearrange("p (f two) -> p f two", f=512, two=2)
                vn = on_[:].rearrange("p (f two) -> p f two", f=512, two=2)
                nc.scalar.copy(vp[:, :, 0], pr)
                nc.vector.tensor_copy(vp[:, :, 1], pi)
                nc.scalar.mul(vn[:, :, 0], pr, -1.0)
                nc.scalar.mul(vn[:, :, 1], pi, -1.0)
                c0 = u * 1024
                nc.sync.dma_start(y_d[r0:r0 + 128, c0:c0 + 1024], op_[:])
                nc.sync.dma_start(
                    y_d[r0:r0 + 128, N + c0:N + c0 + 1024], on_[:])

    nc.compile()
    return nc


def kernel(x: np.ndarray, weights: np.ndarray) -> np.ndarray:
    x = np.ascontiguousarray(np.asarray(x, dtype=np.float32))
    w = np.asarray(weights, dtype=np.float32)
    if "nc" not in _CACHE:
        _CACHE["nc"] = _build_program()
    nc = _CACHE["nc"]
    wts, dgt, cwf = _host_consts(w)
    in_maps = [
        {"x": x[ci * B_CORE:(ci + 1) * B_CORE],
         "wts": wts, "dgt": dgt, "cwf": cwf}
        for ci in range(NCORES)
    ]
    res = run_bass_kernel_spmd(nc, in_maps, list(range(NCORES)))
    _CACHE["last_results"] = res
    out = np.concatenate([res.results[ci]["y"] for ci in range(NCORES)], axis=0)
    return out.view(np.complex64)


# revision 15
# speedup vs baseline: 1.1327x; 1.1327x over previous
"""Trainium2 Bass kernel for nn_CustomLinear (learned-twiddle butterfly net).

Math: reference pads x [2048,4096] to [2048,8192], half-swaps (XOR N/2), then
13 radix-2 butterfly stages with learned twiddles.  After the half-swap the
lower half of each row is zero, so the transform reduces to a 4096-point
network on the nonzero part followed by out = [t, -t].

This version:
  - stages 1..9 folded into one PE matmul phase: out blocks grouped into 4
    parity classes q = j mod 4, each out block = sum of 4 fp16 [128,128]
    complex matmuls over the 4 input blocks of its group.
  - stages 10..12: elementwise butterflies in fp16, transposed layout
    [e=partition, (m,r)=free]; twiddles are per-partition scalars (st10/st11)
    or broadcast fp16 tiles (st12).
  - stage 13 + un-transpose fused on PE: transpose-matmuls with DIAGONAL fp16
    weight tiles (diag(c_j), diag(s_j)) accumulate t^T = (C z_r - S z_i)^T
    and (S z_r + C z_i)^T directly into PSUM (fp16), 1 cycle/row.
  - interleave re/im + negate (-t half) via strided PSUM->SBUF copies, then
    contiguous DMA to HBM.

Sharding: pure data parallel, batch 2048 -> 8 cores x 256 rows.
"""
import numpy as np
from contextlib import ExitStack

import concourse.bacc as bacc
import concourse.mybir as mybir
from concourse.tile import TileContext
from concourse.bass_utils import run_bass_kernel_spmd

N = 8192
B = 2048
IN_F = 4096
NCORES = 8
B_CORE = B // NCORES          # 256 rows per core
NTILES = B_CORE // 128        # 2 row-tiles of 128 rows
F32 = mybir.dt.float32
F16 = mybir.dt.float16

# cwf (fp32 [128,160]) column layout
_ID = 0                        # identity 128 cols
_S10 = 128                     # st10: 128 + q*2 + {c,s}
_S11 = 136                     # st11: 136 + (q*2+u)*2 + {c,s}
CWF_W = 160
# dgt (fp16 [128, 12288]): T13 broadcast tiles (q*2+{c,s})*1024 + m*128 ;
# T12 broadcast tiles at 8192 + (q*2+cs)*512
_T12 = 8192
DGT_W = 12288
# wts (fp16 [128,4096]): ((q*4+jp)*2+{re,im})*128
WTS_W = 4096

_CACHE = {}


def _stage_tw(s, w):
    step = 1 << s
    half = step >> 1
    k = np.arange(half) * (N // step)
    ang = (-2.0 * np.pi / N) * k.astype(np.float64) * w[k].astype(np.float64)
    return np.exp(1j * ang)


def _host_consts(w):
    M = np.eye(128, dtype=np.complex128)
    for s in range(1, 8):
        step = 1 << s
        half = step >> 1
        tw = _stage_tw(s, w)
        Bm = np.zeros((step, step), np.complex128)
        Bm[:half, :half] = np.eye(half)
        Bm[:half, half:] = np.diag(tw)
        Bm[half:, :half] = np.eye(half)
        Bm[half:, half:] = -np.diag(tw)
        M = np.kron(np.eye(128 // step), Bm) @ M

    tw8, tw9, tw10, tw11, tw12, tw13 = [_stage_tw(s, w) for s in range(8, 14)]
    e = np.arange(128)
    T8 = tw8[e]
    T9 = {jr: tw9[jr * 128 + e] for jr in (0, 1)}

    wts = np.zeros((128, WTS_W), np.float16)
    for q in range(4):
        jr9 = q % 2
        s9 = 1.0 if q < 2 else -1.0
        s8 = 1.0 if q % 2 == 0 else -1.0
        Ws = [M,
              s8 * T8[:, None] * M,
              s9 * T9[jr9][:, None] * M,
              s8 * s9 * (T9[jr9] * T8)[:, None] * M]
        for jp in range(4):
            WT = Ws[jp].T          # lhsT[e_in, e_out]
            c0 = ((q * 4 + jp) * 2) * 128
            wts[:, c0:c0 + 128] = WT.real.astype(np.float16)
            wts[:, c0 + 128:c0 + 256] = WT.imag.astype(np.float16)

    dgt = np.zeros((128, DGT_W), np.float16)
    for q in range(4):
        for m in range(8):
            d = tw13[(q + 4 * m) * 128 + e]
            c0 = (q * 2) * 1024 + m * 128
            dgt[:, c0:c0 + 128] = \
                np.repeat(d.real.astype(np.float16)[:, None], 128, 1)
            c0 = (q * 2 + 1) * 1024 + m * 128
            dgt[:, c0:c0 + 128] = \
                np.repeat(d.imag.astype(np.float16)[:, None], 128, 1)
    for q in range(4):
        for m in range(4):
            t = tw12[(q + 4 * m) * 128 + e]
            dgt[:, _T12 + (q * 2) * 512 + m * 128:
                _T12 + (q * 2) * 512 + (m + 1) * 128] = \
                np.repeat(t.real.astype(np.float16)[:, None], 128, 1)
            dgt[:, _T12 + (q * 2 + 1) * 512 + m * 128:
                _T12 + (q * 2 + 1) * 512 + (m + 1) * 128] = \
                np.repeat(t.imag.astype(np.float16)[:, None], 128, 1)

    cwf = np.zeros((128, CWF_W), np.float32)
    cwf[:, _ID:_ID + 128] = np.eye(128, dtype=np.float32)
    for q in range(4):
        t = tw10[q * 128 + e]
        cwf[:, _S10 + 2 * q] = t.real.astype(np.float32)
        cwf[:, _S10 + 2 * q + 1] = t.imag.astype(np.float32)
        for u in (0, 1):
            t = tw11[(q + 4 * u) * 128 + e]
            cwf[:, _S11 + (q * 2 + u) * 2] = t.real.astype(np.float32)
            cwf[:, _S11 + (q * 2 + u) * 2 + 1] = t.imag.astype(np.float32)
    return wts, dgt, cwf


def _build_program():
    nc = bacc.Bacc("TRN2", target_bir_lowering=False, debug=False)
    x_d = nc.dram_tensor("x", [B_CORE, IN_F], F32, kind="ExternalInput").ap()
    wts_d = nc.dram_tensor("wts", [128, WTS_W], F16, kind="ExternalInput").ap()
    dgt_d = nc.dram_tensor("dgt", [128, DGT_W], F16, kind="ExternalInput").ap()
    cwf_d = nc.dram_tensor("cwf", [128, CWF_W], F32, kind="ExternalInput").ap()
    y_d = nc.dram_tensor("y", [B_CORE, 2 * N], F32, kind="ExternalOutput").ap()

    AL = mybir.AluOpType

    with TileContext(nc) as tc, ExitStack() as ctx:
        cpool = ctx.enter_context(tc.tile_pool(name="const", bufs=1))
        xpool = ctx.enter_context(tc.tile_pool(name="xin", bufs=2))
        xtpool = ctx.enter_context(tc.tile_pool(name="xt", bufs=2))
        zpool = ctx.enter_context(tc.tile_pool(name="z", bufs=2))
        spool = ctx.enter_context(tc.tile_pool(name="scr", bufs=4))
        opool = ctx.enter_context(tc.tile_pool(name="out", bufs=2))
        ps_t = ctx.enter_context(tc.tile_pool(name="ps_t", bufs=2, space="PSUM"))
        ps_z = ctx.enter_context(tc.tile_pool(name="ps_z", bufs=4, space="PSUM"))
        ps_o = ctx.enter_context(tc.tile_pool(name="ps_o", bufs=2, space="PSUM"))

        wts = cpool.tile([128, WTS_W], F16)
        dgt = cpool.tile([128, DGT_W], F16)
        cwf = cpool.tile([128, CWF_W], F32)
        nc.sync.dma_start(cwf[:], cwf_d[:])
        ident = cwf[:, _ID:_ID + 128]
        idf16 = cpool.tile([128, 128], F16)

        xins = []
        for ti in range(NTILES):
            r0 = ti * 128
            xin = xpool.tile([128, IN_F], F32, name=f"xin{ti}", tag="xin")
            nc.sync.dma_start(xin[:, :2048], x_d[r0:r0 + 128, :2048])
            nc.sync.dma_start(xin[:, 2048:], x_d[r0:r0 + 128, 2048:])
            xins.append(xin)
            if ti == 0:
                nc.sync.dma_start(wts[:], wts_d[:])
        nc.sync.dma_start(dgt[:], dgt_d[:])
        nc.scalar.copy(idf16[:], ident)

        def wtile(q, jp, cs):
            c0 = ((q * 4 + jp) * 2 + cs) * 128
            return wts[:, c0:c0 + 128]

        def t13t(q, cs):
            c0 = (q * 2 + cs) * 1024
            return dgt[:, c0:c0 + 1024]

        def t12t(q, cs):
            c0 = _T12 + (q * 2 + cs) * 512
            return dgt[:, c0:c0 + 512]

        # ---- per-tile: transpose + phase A ----
        ZR, ZI = [None] * NTILES, [None] * NTILES

        def loadA(ti):
            xin = xins[ti]
            xt = xtpool.tile([128, IN_F], F16, name=f"xt{ti}", tag="xt")
            for bq in range(8):
                pt = ps_t.tile([128, 512], F32, tag="pt")
                for k in range(4):
                    j = bq * 4 + k
                    nc.tensor.transpose(pt[:, k * 128:(k + 1) * 128],
                                        xin[:, j * 128:(j + 1) * 128], ident)
                if bq % 4 == 3:
                    nc.vector.tensor_copy(xt[:, bq * 512:(bq + 1) * 512],
                                          pt[:])
                else:
                    nc.scalar.copy(xt[:, bq * 512:(bq + 1) * 512], pt[:])

            xtv = xt[:].rearrange("p (g v c) -> p g v c", g=8, v=4, c=128)
            zr = [None] * 4
            zi = [None] * 4
            for q in range(4):
                zr[q] = zpool.tile([128, 1024], F16, name=f"zr{ti}{q}",
                                   tag=f"zr{q}")
                zi[q] = zpool.tile([128, 1024], F16, name=f"zi{ti}{q}",
                                   tag=f"zi{q}")
                for h in range(2):
                    sl = slice(h * 512, (h + 1) * 512)
                    for cs, dst in ((0, zr[q]), (1, zi[q])):
                        pz = ps_z.tile([128, 512], F32, tag="pz")
                        for jp in range(4):
                            rhs = xtv[:, 4 * h:4 * h + 4, jp, :]
                            nc.tensor.matmul(pz[:], wtile(q, jp, cs), rhs,
                                             start=(jp == 0), stop=(jp == 3))
                        if (q + h + cs) % 4 == 3:
                            nc.vector.tensor_copy(dst[:, sl], pz[:])
                        else:
                            nc.scalar.copy(dst[:, sl], pz[:])
            ZR[ti] = zr
            ZI[ti] = zi

        # ---- stages 10..13: stage-major, interleaved over (tile, class) ----
        def st10(ti, q):
            zr, zi = ZR[ti][q], ZI[ti][q]
            c10 = cwf[:, _S10 + 2 * q:_S10 + 2 * q + 1]
            s10 = cwf[:, _S10 + 2 * q + 1:_S10 + 2 * q + 2]
            vr = zr[:].rearrange("p (t two c) -> p t two c", two=2, c=128)
            vi = zi[:].rearrange("p (t two c) -> p t two c", two=2, c=128)
            lo_r, hi_r = vr[:, :, 0, :], vr[:, :, 1, :]
            lo_i, hi_i = vi[:, :, 0, :], vi[:, :, 1, :]
            m1 = spool.tile([128, 512], F16, tag="m1")
            m2 = spool.tile([128, 512], F16, tag="m2")
            tr = spool.tile([128, 512], F16, tag="tr")
            tmi = spool.tile([128, 512], F16, tag="tmi")
            w1 = m1[:].rearrange("p (t c) -> p t c", t=4, c=128)
            w2 = m2[:].rearrange("p (t c) -> p t c", t=4, c=128)
            wr = tr[:].rearrange("p (t c) -> p t c", t=4, c=128)
            wi = tmi[:].rearrange("p (t c) -> p t c", t=4, c=128)
            nc.scalar.mul(w1, hi_i, s10)
            nc.scalar.mul(w2, hi_i, c10)
            nc.vector.scalar_tensor_tensor(
                wr, hi_r, c10, w1, op0=AL.mult, op1=AL.subtract)
            nc.vector.scalar_tensor_tensor(
                wi, hi_r, s10, w2, op0=AL.mult, op1=AL.add)
            nc.gpsimd.tensor_tensor(hi_r, lo_r, wr, op=AL.subtract)
            nc.vector.tensor_tensor(hi_i, lo_i, wi, op=AL.subtract)
            nc.vector.tensor_tensor(lo_r, lo_r, wr, op=AL.add)
            nc.vector.tensor_tensor(lo_i, lo_i, wi, op=AL.add)

        def st11(ti, q):
            zr, zi = ZR[ti][q], ZI[ti][q]
            vr = zr[:].rearrange("p (g v c) -> p g v c", g=2, v=4, c=128)
            vi = zi[:].rearrange("p (g v c) -> p g v c", g=2, v=4, c=128)
            for u in (0, 1):
                c11 = cwf[:, _S11 + (q * 2 + u) * 2:
                          _S11 + (q * 2 + u) * 2 + 1]
                s11 = cwf[:, _S11 + (q * 2 + u) * 2 + 1:
                          _S11 + (q * 2 + u) * 2 + 2]
                lo_r, hi_r = vr[:, :, u, :], vr[:, :, u + 2, :]
                lo_i, hi_i = vi[:, :, u, :], vi[:, :, u + 2, :]
                m1 = spool.tile([128, 256], F16, tag="m1b")
                m2 = spool.tile([128, 256], F16, tag="m2b")
                tr = spool.tile([128, 256], F16, tag="trb")
                tmi = spool.tile([128, 256], F16, tag="timb")
                w1 = m1[:].rearrange("p (g c) -> p g c", g=2, c=128)
                w2 = m2[:].rearrange("p (g c) -> p g c", g=2, c=128)
                wr = tr[:].rearrange("p (g c) -> p g c", g=2, c=128)
                wi = tmi[:].rearrange("p (g c) -> p g c", g=2, c=128)
                nc.scalar.mul(w1, hi_i, s11)
                nc.scalar.mul(w2, hi_i, c11)
                nc.vector.scalar_tensor_tensor(
                    wr, hi_r, c11, w1, op0=AL.mult, op1=AL.subtract)
                nc.vector.scalar_tensor_tensor(
                    wi, hi_r, s11, w2, op0=AL.mult, op1=AL.add)
                nc.gpsimd.tensor_tensor(hi_r, lo_r, wr, op=AL.subtract)
                nc.gpsimd.tensor_tensor(hi_i, lo_i, wi, op=AL.subtract)
                nc.vector.tensor_tensor(lo_r, lo_r, wr, op=AL.add)
                nc.vector.tensor_tensor(lo_i, lo_i, wi, op=AL.add)

        def st12(ti, q):
            zr, zi = ZR[ti][q], ZI[ti][q]
            lo_r, hi_r = zr[:, :512], zr[:, 512:]
            lo_i, hi_i = zi[:, :512], zi[:, 512:]
            u1 = spool.tile([128, 512], F16, tag="u1")
            u2 = spool.tile([128, 512], F16, tag="u2")
            tr = spool.tile([128, 512], F16, tag="tr2")
            tmi = spool.tile([128, 512], F16, tag="tim2")
            nc.vector.tensor_tensor(u1[:], hi_r, t12t(q, 0), op=AL.mult)
            nc.vector.tensor_tensor(u2[:], hi_i, t12t(q, 1), op=AL.mult)
            nc.vector.tensor_tensor(tr[:], u1[:], u2[:], op=AL.subtract)
            nc.gpsimd.tensor_tensor(u1[:], hi_r, t12t(q, 1), op=AL.mult)
            nc.vector.tensor_tensor(u2[:], hi_i, t12t(q, 0), op=AL.mult)
            nc.gpsimd.tensor_tensor(tmi[:], u1[:], u2[:], op=AL.add)
            nc.gpsimd.tensor_tensor(hi_r, lo_r, tr[:], op=AL.subtract)
            nc.vector.tensor_tensor(hi_i, lo_i, tmi[:], op=AL.subtract)
            nc.vector.tensor_tensor(lo_r, lo_r, tr[:], op=AL.add)
            nc.vector.tensor_tensor(lo_i, lo_i, tmi[:], op=AL.add)

        def st13(ti, q):
            # in place: zr <- Re(tw13*z), zi <- Im(tw13*z)
            zr, zi = ZR[ti][q], ZI[ti][q]
            u1 = spool.tile([128, 1024], F16, tag="su1")
            u2 = spool.tile([128, 1024], F16, tag="su2")
            u3 = spool.tile([128, 1024], F16, tag="su3")
            u4 = spool.tile([128, 1024], F16, tag="su4")
            nc.vector.tensor_tensor(u1[:], zr[:], t13t(q, 0), op=AL.mult)
            nc.vector.tensor_tensor(u2[:], zi[:], t13t(q, 1), op=AL.mult)
            nc.gpsimd.tensor_tensor(u3[:], zr[:], t13t(q, 1), op=AL.mult)
            nc.vector.tensor_tensor(u4[:], zi[:], t13t(q, 0), op=AL.mult)
            nc.vector.tensor_tensor(zr[:], u1[:], u2[:], op=AL.subtract)
            nc.vector.tensor_tensor(zi[:], u3[:], u4[:], op=AL.add)

        def outstage(ti):
            r0 = ti * 128
            tre, tim = ZR[ti], ZI[ti]
            for u in range(8):
                po = ps_o.tile([128, 1024], F16, tag="po")
                pr, pi = po[:, :512], po[:, 512:]
                for q in range(4):
                    ms = slice(u * 128, (u + 1) * 128)
                    os_ = slice(q * 128, (q + 1) * 128)
                    nc.tensor.transpose(pr[:, os_], tre[q][:, ms], idf16[:])
                    nc.tensor.transpose(pi[:, os_], tim[q][:, ms], idf16[:])
                op_ = opool.tile([128, 1024], F32, tag="op")
                on_ = opool.tile([128, 1024], F32, tag="on")
                vp = op_[:].rearrange("p (f two) -> p f two", f=512, two=2)
                vn = on_[:].rearrange("p (f two) -> p f two", f=512, two=2)
                nc.scalar.copy(vp[:, :, 0], pr)
                nc.vector.tensor_copy(vp[:, :, 1], pi)
                nc.scalar.mul(vn[:, :, 0], pr, -1.0)
                nc.scalar.mul(vn[:, :, 1], pi, -1.0)
                c0 = u * 1024
                nc.sync.dma_start(y_d[r0:r0 + 128, c0:c0 + 1024], op_[:])
                nc.sync.dma_start(
                    y_d[r0:r0 + 128, N + c0:N + c0 + 1024], on_[:])

        def stages(ti):
            for q in range(4):
                st10(ti, q)
                st11(ti, q)
                st12(ti, q)
                st13(ti, q)

        loadA(0)
        stages(0)
        loadA(1)
        outstage(0)
        stages(1)
        outstage(1)

    nc.compile()
    return nc


def kernel(x: np.ndarray, weights: np.ndarray) -> np.ndarray:
    x = np.ascontiguousarray(np.asarray(x, dtype=np.float32))
    w = np.asarray(weights, dtype=np.float32)
    if "nc" not in _CACHE:
        _CACHE["nc"] = _build_program()
    nc = _CACHE["nc"]
    wts, dgt, cwf = _host_consts(w)
    in_maps = [
        {"x": x[ci * B_CORE:(ci + 1) * B_CORE],
         "wts": wts, "dgt": dgt, "cwf": cwf}
        for ci in range(NCORES)
    ]
    res = run_bass_kernel_spmd(nc, in_maps, list(range(NCORES)))
    _CACHE["last_results"] = res
    out = np.concatenate([res.results[ci]["y"] for ci in range(NCORES)], axis=0)
    return out.view(np.complex64)


# revision 16
# speedup vs baseline: 1.1777x; 1.0397x over previous
"""Trainium2 Bass kernel for nn_CustomLinear (learned-twiddle butterfly net).

Math: reference pads x [2048,4096] to [2048,8192], half-swaps (XOR N/2), then
13 radix-2 butterfly stages with learned twiddles.  After the half-swap the
lower half of each row is zero, so the transform reduces to a 4096-point
network on the nonzero part followed by out = [t, -t].

This version:
  - stages 1..9 folded into one PE matmul phase: out blocks grouped into 4
    parity classes q = j mod 4, each out block = sum of 4 fp16 [128,128]
    complex matmuls over the 4 input blocks of its group.
  - stages 10..12: elementwise butterflies in fp16, transposed layout
    [e=partition, (m,r)=free]; twiddles are per-partition scalars (st10/st11)
    or broadcast fp16 tiles (st12).
  - stage 13 + un-transpose fused on PE: transpose-matmuls with DIAGONAL fp16
    weight tiles (diag(c_j), diag(s_j)) accumulate t^T = (C z_r - S z_i)^T
    and (S z_r + C z_i)^T directly into PSUM (fp16), 1 cycle/row.
  - interleave re/im + negate (-t half) via strided PSUM->SBUF copies, then
    contiguous DMA to HBM.

Sharding: pure data parallel, batch 2048 -> 8 cores x 256 rows.
"""
import numpy as np
from contextlib import ExitStack

import concourse.bacc as bacc
import concourse.mybir as mybir
from concourse.tile import TileContext
from concourse.bass_utils import run_bass_kernel_spmd

N = 8192
B = 2048
IN_F = 4096
NCORES = 8
B_CORE = B // NCORES          # 256 rows per core
NTILES = B_CORE // 128        # 2 row-tiles of 128 rows
F32 = mybir.dt.float32
F16 = mybir.dt.float16

# cwf (fp32 [128,160]) column layout
_ID = 0                        # identity 128 cols
_S10 = 128                     # st10: 128 + q*2 + {c,s}
_S11 = 136                     # st11: 136 + (q*2+u)*2 + {c,s}
CWF_W = 160
# dgt (fp16 [128, 12288]): T13 broadcast tiles (q*2+{c,s})*1024 + m*128 ;
# T12 broadcast tiles at 8192 + (q*2+cs)*512
_T12 = 8192
DGT_W = 12288
# wts (fp16 [128,4096]): ((q*4+jp)*2+{re,im})*128
WTS_W = 4096

_CACHE = {}


def _stage_tw(s, w):
    step = 1 << s
    half = step >> 1
    k = np.arange(half) * (N // step)
    ang = (-2.0 * np.pi / N) * k.astype(np.float64) * w[k].astype(np.float64)
    return np.exp(1j * ang)


def _host_consts(w):
    M = np.eye(128, dtype=np.complex128)
    for s in range(1, 8):
        step = 1 << s
        half = step >> 1
        tw = _stage_tw(s, w)
        Bm = np.zeros((step, step), np.complex128)
        Bm[:half, :half] = np.eye(half)
        Bm[:half, half:] = np.diag(tw)
        Bm[half:, :half] = np.eye(half)
        Bm[half:, half:] = -np.diag(tw)
        M = np.kron(np.eye(128 // step), Bm) @ M

    tw8, tw9, tw10, tw11, tw12, tw13 = [_stage_tw(s, w) for s in range(8, 14)]
    e = np.arange(128)
    T8 = tw8[e]
    T9 = {jr: tw9[jr * 128 + e] for jr in (0, 1)}

    wts = np.zeros((128, WTS_W), np.float16)
    for q in range(4):
        jr9 = q % 2
        s9 = 1.0 if q < 2 else -1.0
        s8 = 1.0 if q % 2 == 0 else -1.0
        Ws = [M,
              s8 * T8[:, None] * M,
              s9 * T9[jr9][:, None] * M,
              s8 * s9 * (T9[jr9] * T8)[:, None] * M]
        for jp in range(4):
            WT = Ws[jp].T          # lhsT[e_in, e_out]
            c0 = ((q * 4 + jp) * 2) * 128
            wts[:, c0:c0 + 128] = WT.real.astype(np.float16)
            wts[:, c0 + 128:c0 + 256] = WT.imag.astype(np.float16)

    dgt = np.zeros((128, DGT_W), np.float16)
    for q in range(4):
        for m in range(8):
            d = tw13[(q + 4 * m) * 128 + e]
            c0 = (q * 2) * 1024 + m * 128
            dgt[:, c0:c0 + 128] = \
                np.repeat(d.real.astype(np.float16)[:, None], 128, 1)
            c0 = (q * 2 + 1) * 1024 + m * 128
            dgt[:, c0:c0 + 128] = \
                np.repeat(d.imag.astype(np.float16)[:, None], 128, 1)
    for q in range(4):
        for m in range(4):
            t = tw12[(q + 4 * m) * 128 + e]
            dgt[:, _T12 + (q * 2) * 512 + m * 128:
                _T12 + (q * 2) * 512 + (m + 1) * 128] = \
                np.repeat(t.real.astype(np.float16)[:, None], 128, 1)
            dgt[:, _T12 + (q * 2 + 1) * 512 + m * 128:
                _T12 + (q * 2 + 1) * 512 + (m + 1) * 128] = \
                np.repeat(t.imag.astype(np.float16)[:, None], 128, 1)

    cwf = np.zeros((128, CWF_W), np.float32)
    cwf[:, _ID:_ID + 128] = np.eye(128, dtype=np.float32)
    for q in range(4):
        t = tw10[q * 128 + e]
        cwf[:, _S10 + 2 * q] = t.real.astype(np.float32)
        cwf[:, _S10 + 2 * q + 1] = t.imag.astype(np.float32)
        for u in (0, 1):
            t = tw11[(q + 4 * u) * 128 + e]
            cwf[:, _S11 + (q * 2 + u) * 2] = t.real.astype(np.float32)
            cwf[:, _S11 + (q * 2 + u) * 2 + 1] = t.imag.astype(np.float32)
    return wts, dgt, cwf


def _build_program():
    nc = bacc.Bacc("TRN2", target_bir_lowering=False, debug=False)
    x_d = nc.dram_tensor("x", [B_CORE, IN_F], F32, kind="ExternalInput").ap()
    wts_d = nc.dram_tensor("wts", [128, WTS_W], F16, kind="ExternalInput").ap()
    dgt_d = nc.dram_tensor("dgt", [128, DGT_W], F16, kind="ExternalInput").ap()
    cwf_d = nc.dram_tensor("cwf", [128, CWF_W], F32, kind="ExternalInput").ap()
    y_d = nc.dram_tensor("y", [B_CORE, 2 * N], F32, kind="ExternalOutput").ap()

    AL = mybir.AluOpType

    with TileContext(nc) as tc, ExitStack() as ctx:
        cpool = ctx.enter_context(tc.tile_pool(name="const", bufs=1))
        xpool = ctx.enter_context(tc.tile_pool(name="xin", bufs=2))
        xtpool = ctx.enter_context(tc.tile_pool(name="xt", bufs=2))
        zpool = ctx.enter_context(tc.tile_pool(name="z", bufs=2))
        spool = ctx.enter_context(tc.tile_pool(name="scr", bufs=4))
        opool = ctx.enter_context(tc.tile_pool(name="out", bufs=2))
        ps_t = ctx.enter_context(tc.tile_pool(name="ps_t", bufs=2, space="PSUM"))
        ps_z = ctx.enter_context(tc.tile_pool(name="ps_z", bufs=4, space="PSUM"))
        ps_o = ctx.enter_context(tc.tile_pool(name="ps_o", bufs=2, space="PSUM"))

        wts = cpool.tile([128, WTS_W], F16)
        dgt = cpool.tile([128, DGT_W], F16)
        cwf = cpool.tile([128, CWF_W], F32)
        nc.sync.dma_start(cwf[:], cwf_d[:])
        ident = cwf[:, _ID:_ID + 128]
        idf16 = cpool.tile([128, 128], F16)

        xins = []
        for ti in range(NTILES):
            r0 = ti * 128
            xin = xpool.tile([128, IN_F], F32, name=f"xin{ti}", tag="xin")
            nc.sync.dma_start(xin[:, :2048], x_d[r0:r0 + 128, :2048])
            nc.sync.dma_start(xin[:, 2048:], x_d[r0:r0 + 128, 2048:])
            xins.append(xin)
            if ti == 0:
                nc.sync.dma_start(wts[:], wts_d[:])
        nc.sync.dma_start(dgt[:], dgt_d[:])
        nc.scalar.copy(idf16[:], ident)

        def wtile(q, jp, cs):
            c0 = ((q * 4 + jp) * 2 + cs) * 128
            return wts[:, c0:c0 + 128]

        def t13t(q, cs):
            c0 = (q * 2 + cs) * 1024
            return dgt[:, c0:c0 + 1024]

        def t12t(q, cs):
            c0 = _T12 + (q * 2 + cs) * 512
            return dgt[:, c0:c0 + 512]

        # ---- per-tile: transpose + phase A ----
        ZR, ZI = [None] * NTILES, [None] * NTILES

        def loadA(ti):
            xin = xins[ti]
            xt = xtpool.tile([128, IN_F], F16, name=f"xt{ti}", tag="xt")
            for bq in range(8):
                pt = ps_t.tile([128, 512], F32, tag="pt")
                for k in range(4):
                    j = bq * 4 + k
                    nc.tensor.transpose(pt[:, k * 128:(k + 1) * 128],
                                        xin[:, j * 128:(j + 1) * 128], ident)
                nc.scalar.copy(xt[:, bq * 512:(bq + 1) * 512], pt[:])

            xtv = xt[:].rearrange("p (g v c) -> p g v c", g=8, v=4, c=128)
            zr = [None] * 4
            zi = [None] * 4
            for q in range(4):
                zr[q] = zpool.tile([128, 1024], F16, name=f"zr{ti}{q}",
                                   tag=f"zr{q}")
                zi[q] = zpool.tile([128, 1024], F16, name=f"zi{ti}{q}",
                                   tag=f"zi{q}")
                for h in range(2):
                    sl = slice(h * 512, (h + 1) * 512)
                    for cs, dst in ((0, zr[q]), (1, zi[q])):
                        pz = ps_z.tile([128, 512], F32, tag="pz")
                        for jp in range(4):
                            rhs = xtv[:, 4 * h:4 * h + 4, jp, :]
                            nc.tensor.matmul(pz[:], wtile(q, jp, cs), rhs,
                                             start=(jp == 0), stop=(jp == 3))
                        nc.scalar.copy(dst[:, sl], pz[:])
            ZR[ti] = zr
            ZI[ti] = zi

        # ---- stages 10..13: stage-major, interleaved over (tile, class) ----
        def st10(ti, q):
            zr, zi = ZR[ti][q], ZI[ti][q]
            c10 = cwf[:, _S10 + 2 * q:_S10 + 2 * q + 1]
            s10 = cwf[:, _S10 + 2 * q + 1:_S10 + 2 * q + 2]
            vr = zr[:].rearrange("p (t two c) -> p t two c", two=2, c=128)
            vi = zi[:].rearrange("p (t two c) -> p t two c", two=2, c=128)
            lo_r, hi_r = vr[:, :, 0, :], vr[:, :, 1, :]
            lo_i, hi_i = vi[:, :, 0, :], vi[:, :, 1, :]
            m1 = spool.tile([128, 512], F16, tag="m1", bufs=6)
            m2 = spool.tile([128, 512], F16, tag="m2", bufs=6)
            tr = spool.tile([128, 512], F16, tag="tr")
            tmi = spool.tile([128, 512], F16, tag="tmi")
            w1 = m1[:].rearrange("p (t c) -> p t c", t=4, c=128)
            w2 = m2[:].rearrange("p (t c) -> p t c", t=4, c=128)
            wr = tr[:].rearrange("p (t c) -> p t c", t=4, c=128)
            wi = tmi[:].rearrange("p (t c) -> p t c", t=4, c=128)
            nc.scalar.mul(w1, hi_i, s10)
            nc.scalar.mul(w2, hi_i, c10)
            nc.vector.scalar_tensor_tensor(
                wr, hi_r, c10, w1, op0=AL.mult, op1=AL.subtract)
            nc.vector.scalar_tensor_tensor(
                wi, hi_r, s10, w2, op0=AL.mult, op1=AL.add)
            nc.gpsimd.tensor_tensor(hi_r, lo_r, wr, op=AL.subtract)
            nc.vector.tensor_tensor(hi_i, lo_i, wi, op=AL.subtract)
            nc.vector.tensor_tensor(lo_r, lo_r, wr, op=AL.add)
            nc.vector.tensor_tensor(lo_i, lo_i, wi, op=AL.add)

        def st11(ti, q):
            zr, zi = ZR[ti][q], ZI[ti][q]
            vr = zr[:].rearrange("p (g v c) -> p g v c", g=2, v=4, c=128)
            vi = zi[:].rearrange("p (g v c) -> p g v c", g=2, v=4, c=128)
            for u in (0, 1):
                c11 = cwf[:, _S11 + (q * 2 + u) * 2:
                          _S11 + (q * 2 + u) * 2 + 1]
                s11 = cwf[:, _S11 + (q * 2 + u) * 2 + 1:
                          _S11 + (q * 2 + u) * 2 + 2]
                lo_r, hi_r = vr[:, :, u, :], vr[:, :, u + 2, :]
                lo_i, hi_i = vi[:, :, u, :], vi[:, :, u + 2, :]
                m1 = spool.tile([128, 256], F16, tag="m1b")
                m2 = spool.tile([128, 256], F16, tag="m2b")
                tr = spool.tile([128, 256], F16, tag="trb")
                tmi = spool.tile([128, 256], F16, tag="timb")
                w1 = m1[:].rearrange("p (g c) -> p g c", g=2, c=128)
                w2 = m2[:].rearrange("p (g c) -> p g c", g=2, c=128)
                wr = tr[:].rearrange("p (g c) -> p g c", g=2, c=128)
                wi = tmi[:].rearrange("p (g c) -> p g c", g=2, c=128)
                nc.scalar.mul(w1, hi_i, s11)
                nc.scalar.mul(w2, hi_i, c11)
                nc.vector.scalar_tensor_tensor(
                    wr, hi_r, c11, w1, op0=AL.mult, op1=AL.subtract)
                nc.vector.scalar_tensor_tensor(
                    wi, hi_r, s11, w2, op0=AL.mult, op1=AL.add)
                nc.gpsimd.tensor_tensor(hi_r, lo_r, wr, op=AL.subtract)
                nc.gpsimd.tensor_tensor(hi_i, lo_i, wi, op=AL.subtract)
                nc.vector.tensor_tensor(lo_r, lo_r, wr, op=AL.add)
                nc.vector.tensor_tensor(lo_i, lo_i, wi, op=AL.add)

        def st12(ti, q):
            zr, zi = ZR[ti][q], ZI[ti][q]
            lo_r, hi_r = zr[:, :512], zr[:, 512:]
            lo_i, hi_i = zi[:, :512], zi[:, 512:]
            u1 = spool.tile([128, 512], F16, tag="u1")
            u2 = spool.tile([128, 512], F16, tag="u2")
            tr = spool.tile([128, 512], F16, tag="tr2")
            tmi = spool.tile([128, 512], F16, tag="tim2")
            nc.vector.tensor_tensor(u1[:], hi_r, t12t(q, 0), op=AL.mult)
            nc.vector.tensor_tensor(u2[:], hi_i, t12t(q, 1), op=AL.mult)
            nc.vector.tensor_tensor(tr[:], u1[:], u2[:], op=AL.subtract)
            nc.gpsimd.tensor_tensor(u1[:], hi_r, t12t(q, 1), op=AL.mult)
            nc.vector.tensor_tensor(u2[:], hi_i, t12t(q, 0), op=AL.mult)
            nc.gpsimd.tensor_tensor(tmi[:], u1[:], u2[:], op=AL.add)
            nc.gpsimd.tensor_tensor(hi_r, lo_r, tr[:], op=AL.subtract)
            nc.vector.tensor_tensor(hi_i, lo_i, tmi[:], op=AL.subtract)
            nc.vector.tensor_tensor(lo_r, lo_r, tr[:], op=AL.add)
            nc.vector.tensor_tensor(lo_i, lo_i, tmi[:], op=AL.add)

        def st13(ti, q):
            # in place: zr <- Re(tw13*z), zi <- Im(tw13*z)
            zr, zi = ZR[ti][q], ZI[ti][q]
            u1 = spool.tile([128, 1024], F16, tag="su1")
            u2 = spool.tile([128, 1024], F16, tag="su2")
            u3 = spool.tile([128, 1024], F16, tag="su3")
            u4 = spool.tile([128, 1024], F16, tag="su4")
            nc.vector.tensor_tensor(u1[:], zr[:], t13t(q, 0), op=AL.mult)
            nc.vector.tensor_tensor(u2[:], zi[:], t13t(q, 1), op=AL.mult)
            nc.gpsimd.tensor_tensor(u3[:], zr[:], t13t(q, 1), op=AL.mult)
            nc.vector.tensor_tensor(u4[:], zi[:], t13t(q, 0), op=AL.mult)
            nc.vector.tensor_tensor(zr[:], u1[:], u2[:], op=AL.subtract)
            nc.vector.tensor_tensor(zi[:], u3[:], u4[:], op=AL.add)

        def outstage(ti):
            r0 = ti * 128
            tre, tim = ZR[ti], ZI[ti]
            for u in range(8):
                po = ps_o.tile([128, 1024], F16, tag="po")
                pr, pi = po[:, :512], po[:, 512:]
                for q in range(4):
                    ms = slice(u * 128, (u + 1) * 128)
                    os_ = slice(q * 128, (q + 1) * 128)
                    nc.tensor.transpose(pr[:, os_], tre[q][:, ms], idf16[:])
                    nc.tensor.transpose(pi[:, os_], tim[q][:, ms], idf16[:])
                op_ = opool.tile([128, 1024], F32, tag="op")
                on_ = opool.tile([128, 1024], F32, tag="on")
                vp = op_[:].rearrange("p (f two) -> p f two", f=512, two=2)
                vn = on_[:].rearrange("p (f two) -> p f two", f=512, two=2)
                nc.scalar.copy(vp[:, :, 0], pr)
                nc.scalar.copy(vp[:, :, 1], pi)
                nc.vector.tensor_scalar_mul(vn[:, :, 0], pr, -1.0)
                nc.scalar.mul(vn[:, :, 1], pi, -1.0)
                c0 = u * 1024
                nc.sync.dma_start(y_d[r0:r0 + 128, c0:c0 + 1024], op_[:])
                nc.sync.dma_start(
                    y_d[r0:r0 + 128, N + c0:N + c0 + 1024], on_[:])

        def stages(ti):
            for q in range(4):
                st10(ti, q)
                st11(ti, q)
                st12(ti, q)
                st13(ti, q)

        loadA(0)
        stages(0)
        loadA(1)
        outstage(0)
        stages(1)
        outstage(1)

    nc.compile()
    return nc


def kernel(x: np.ndarray, weights: np.ndarray) -> np.ndarray:
    x = np.ascontiguousarray(np.asarray(x, dtype=np.float32))
    w = np.asarray(weights, dtype=np.float32)
    if "nc" not in _CACHE:
        _CACHE["nc"] = _build_program()
    nc = _CACHE["nc"]
    wts, dgt, cwf = _host_consts(w)
    in_maps = [
        {"x": x[ci * B_CORE:(ci + 1) * B_CORE],
         "wts": wts, "dgt": dgt, "cwf": cwf}
        for ci in range(NCORES)
    ]
    res = run_bass_kernel_spmd(nc, in_maps, list(range(NCORES)))
    _CACHE["last_results"] = res
    out = np.concatenate([res.results[ci]["y"] for ci in range(NCORES)], axis=0)
    return out.view(np.complex64)
